# revision 34
# baseline (speedup 1.0000x reference)
"""Trainium2 Bass kernel for nn_DimMasking (iterative softmax top-k masking).

Full-input contract: kernel(**inputs) takes the unsharded inputs
(x [8192,640], W1 [640,64], b1 [64], W2 [64,640], b2 [640]) and returns the
full [8192,640] output. Internally: pure data parallel over the batch dim —
8 shards of 1024 rows, one per NeuronCore; MLP weights replicated.

Math: exp-domain reformulation of the reference scan (validated to ~1e-6
absmax-relative vs the fp32 reference in numpy, ~2e-3 on HW splines):
    h  = relu(x@W1 + b1)@W2 + b2
    e  = exp((log(1+eps) - h)/T);  Z = rowsum(e)
    repeat 64x:
        v = (e - Z) * (-1/Z)     # = 1 - softmax-prob, >= 0   [DVE ts]
        w = v^(1/T)              # ln + exp(scale=1/T)        [2 ACT passes]
        e *= w;  Z = rowsum(e)   # fused custom-DVE mul+reduce [DVE]
    out = (exp(T*ln(e) + h) - eps) * x
which is exactly m' = m*(1 - softmax((log(m+eps)-h)/T)) expressed on
e = ((m+eps)^(1/T))*exp(-h/T), so the rowwise softmax never needs a
separate normalize, max-subtract, or per-iteration log of the state.

Per iteration the two 2560-wide ACT transcendental passes per half
(4 total) are the bottleneck; two independent half-streams keep ACT
saturated. ~701us/core predicted by the cost-model timeline sim;
~790us/core measured on HW via in-NEFF loop repetition deltas.
"""

import numpy as np

import concourse.tile as tile
from concourse import bacc, masks, mybir
from concourse.bass_utils import run_bass_kernel_spmd

# Pin the ACT spline-table set to natural_log_exp_and_others (it contains
# every function this kernel uses: Exp, Ln, Copy, Identity, Relu). Without
# this, the table-load-insertion pass alternates between the exp-only and
# ln-only sets, paying a ~1.3us table reload twice per masking iteration.
_orig_get_tables = bacc.get_activation_tables


def _pinned_get_tables(module_arch):
    tables = _orig_get_tables(module_arch)
    combined = tables.get("natural_log_exp_and_others")
    needed = {
        mybir.ActivationFunctionType.Exp,
        mybir.ActivationFunctionType.Ln,
        mybir.ActivationFunctionType.Copy,
        mybir.ActivationFunctionType.Identity,
        mybir.ActivationFunctionType.Relu,
    }
    if not combined or not needed.issubset(combined):
        return tables  # fall back to default selection
    pinned = {}
    for name, fns in tables.items():
        pinned[name] = fns if name == "natural_log_exp_and_others" else set()
    return pinned


import os as _os
PIN_ACT = _os.environ.get("NO_PIN_ACT", "0") != "1"
USE_TTR = int(_os.environ.get("TTR_MODE", "2"))  # 1=isa-ttr 2=custom-dve 0=plain
POOL_TS = _os.environ.get("POOL_TS", "0") == "1"
PER_GROUP_RECIP = _os.environ.get("PGR", "0") == "1"

F32 = mybir.dt.float32
AF = mybir.ActivationFunctionType
OP = mybir.AluOpType

N_CORES = 8
B = 8192
D = 640          # 5 chunks of 128
HID = 64
R = B // N_CORES  # 1024 rows per core
P = 128
G = R // P        # 8 row-groups per core
DC = D // P       # 5 dim-chunks
N_ITER = 64
TEMP = 0.07
EPS = 1e-7
C0 = float(np.log1p(np.float32(EPS)) / np.float32(TEMP))
INV_T = float(np.float32(1.0) / np.float32(TEMP))

# module-level cache: build/compile once per process
_CACHE = {}


def _build_nc(n_iter=N_ITER, num_devices=N_CORES, taps=(), reps=1):
    nc = bacc.Bacc(
        "TRN2",
        target_bir_lowering=False,
        debug=False,
        enable_asserts=False,
        num_devices=num_devices,
    )
    x_d = nc.dram_tensor("x", [R, D], F32, kind="ExternalInput").ap()
    w1_d = nc.dram_tensor("w1", [D, HID], F32, kind="ExternalInput").ap()
    b1_d = nc.dram_tensor("b1", [HID, 1], F32, kind="ExternalInput").ap()
    w2b_d = nc.dram_tensor("w2b", [HID + 1, D], F32, kind="ExternalInput").ap()
    out_d = nc.dram_tensor("out", [R, D], F32, kind="ExternalOutput").ap()
    tap_aps = {
        name: nc.dram_tensor(f"tap_{name}", [R, D], F32, kind="ExternalOutput").ap()
        for name in taps
    }

    with tile.TileContext(nc) as tc:
        _emit(tc, out_d, x_d, w1_d, b1_d, w2b_d, n_iter=n_iter, tap_aps=tap_aps,
              reps=reps)
    # Scope the activation-table pin strictly to compiling OUR module.
    saved = bacc.get_activation_tables
    try:
        if PIN_ACT:
            bacc.get_activation_tables = _pinned_get_tables
        nc.compile()
    finally:
        bacc.get_activation_tables = saved
    return nc


def _dma_out_groups(nc, dram_ap, sbuf_tile):
    for g in range(G):
        nc.sync.dma_start(out=dram_ap[g * P:(g + 1) * P, :], in_=sbuf_tile[:, g, :])


def _emit(tc, out_d, x_d, w1_d, b1_d, w2b_d, n_iter=N_ITER, tap_aps=None,
          reps=1):
    nc = tc.nc
    from contextlib import ExitStack

    ctx = ExitStack()
    with ctx:
        singles = ctx.enter_context(tc.tile_pool(name="singles", bufs=1))
        zpool = ctx.enter_context(tc.tile_pool(name="zpool", bufs=3))

        # persistent SBUF tensors
        xs = singles.tile([P, G, D], F32)    # x, rows-on-partitions
        xt = singles.tile([P, DC, R], F32)   # x transposed (d-on-partitions)
        hs = singles.tile([P, G, D], F32)    # MLP output h
        es = singles.tile([P, G, D], F32)    # state e = exp(logits)
        vs = singles.tile([P, G, D], F32)    # scratch (v, then w)
        ws = singles.tile([P, G, D], F32)    # scratch (L2, finale temps)
        zh_a = singles.tile([P, G // 2], F32)
        zh_b = singles.tile([P, G // 2], F32)
        zhalf = [zh_a, zh_b]
        w1s = singles.tile([P, DC, HID], F32)
        b1s = singles.tile([HID, 1], F32)
        w2bs = singles.tile([HID + 1, D], F32)
        h1r = singles.tile([HID + 1, R], F32)  # relu(x@W1+b1).T with ones row
        ident = singles.tile([P, P], F32)
        c0s = singles.tile([P, 1], F32)        # bias constant C0 for init exp
        nc.vector.memset(c0s[:, :], C0)

        # ---- input DMAs ----
        for g in range(G):
            nc.sync.dma_start(out=xs[:, g, :], in_=x_d[g * P:(g + 1) * P, :])
        nc.sync.dma_start(out=w1s[:, :, :],
                          in_=w1_d.rearrange("(c p) j -> p c j", p=P))
        nc.sync.dma_start(out=b1s[:, :], in_=b1_d[:, :])
        nc.sync.dma_start(out=w2bs[:, :], in_=w2b_d[:, :])

        masks.make_identity(nc, ident[:, :])

        # ---- transpose x: 40 PE transposes of [128,128] blocks.
        # 4 transposes share one PSUM bank, evacuated with a single wide copy
        # (alternating DVE/ACT) to halve evacuation time.
        with tc.tile_pool(name="tp_psum", bufs=3, space="PSUM") as tpp:
            for c in range(DC):
                for gq in range(G // 4):
                    tp = tpp.tile([P, 4 * P], F32)
                    for gj in range(4):
                        g = gq * 4 + gj
                        nc.tensor.transpose(
                            tp[:, gj * P:(gj + 1) * P],
                            xs[:, g, c * P:(c + 1) * P], ident[:, :])
                    dst = xt[:, c, gq * 4 * P:(gq + 1) * 4 * P]
                    if (c + gq) % 2 == 0:
                        nc.vector.tensor_copy(dst, tp[:, :])
                    else:
                        nc.scalar.copy(dst, tp[:, :])

        # ---- MLP matmul 1: h1T[j, r] = sum_d W1[d,j] * xT[d,r] ----
        NH = 2  # split R into halves of 512 (f32 moving max)
        with tc.tile_pool(name="mm1_psum", bufs=2, space="PSUM") as mp1:
            for nh in range(NH):
                ph1 = mp1.tile([HID, R // NH], F32, tag="ph1")
                for c in range(DC):
                    nc.tensor.matmul(
                        ph1[:, :],
                        w1s[:, c, :],
                        xt[:, c, nh * 512:(nh + 1) * 512],
                        start=(c == 0),
                        stop=(c == DC - 1),
                    )
                # relu(+bias) straight out of PSUM
                nc.scalar.activation(
                    h1r[0:HID, nh * 512:(nh + 1) * 512], ph1[:, :],
                    AF.Relu, bias=b1s[:, 0:1], scale=1.0)
        nc.vector.memset(h1r[HID:HID + 1, :], 1.0)

        # ---- MLP matmul 2 + evac: h = h1r.T @ W2b  (bias via ones row) ----
        # Evac does double duty: keep h for the finale, and initialize the
        # loop state e0 = exp(-h/T + C0) with per-group row-sums Z0.
        with tc.tile_pool(name="mm2_psum", bufs=2, space="PSUM") as mp2:
            for g in range(G):
                ph = mp2.tile([P, D], F32, tag="ph")
                lhs = h1r[:, g * P:(g + 1) * P]
                nc.tensor.matmul(ph[:, 0:512], lhs, w2bs[:, 0:512],
                                 start=True, stop=True)
                nc.tensor.matmul(ph[:, 512:D], lhs, w2bs[:, 512:D],
                                 start=True, stop=True)
                nc.vector.tensor_copy(hs[:, g, :], ph[:, :])
                nc.scalar.activation(es[:, g, :], ph[:, :], AF.Exp,
                                     bias=c0s[:, 0:1], scale=-INV_T,
                                     accum_out=zhalf[g // (G // 2)][
                                         :, g % (G // 2):g % (G // 2) + 1])

        if tap_aps is None:
            tap_aps = {}
        if "h" in tap_aps:
            _dma_out_groups(nc, tap_aps["h"], hs)

        # ---- the masking loop (state: e, Z) ----
        HG = G // 2  # 4 groups per half-stream
        for it in range(n_iter * reps):
            for half in range(2):
                g0 = half * HG
                csl = slice(g0, g0 + HG)
                zh = zhalf[half]
                nzh = zpool.tile([P, HG], F32, tag="nzh")
                nr = zpool.tile([P, HG], F32, tag="nr")
                if PER_GROUP_RECIP:
                    # per-group negate+recip+v so the first v tiles are ready
                    # as soon as their own Z lands (shorter DVE->ACT chain)
                    for gi in range(HG):
                        g = g0 + gi
                        nc.vector.tensor_scalar_mul(
                            nzh[:, gi:gi + 1], zh[:, gi:gi + 1], -1.0)
                        nc.vector.reciprocal(
                            nr[:, gi:gi + 1], nzh[:, gi:gi + 1])
                        nc.vector.tensor_scalar(
                            out=vs[:, g, :], in0=es[:, g, :],
                            scalar1=zh[:, gi:gi + 1], scalar2=nr[:, gi:gi + 1],
                            op0=OP.subtract, op1=OP.mult)
                else:
                    nc.vector.tensor_scalar_mul(nzh[:, :], zh[:, :], -1.0)
                    nc.vector.reciprocal(nr[:, :], nzh[:, :])  # -1/Z
                    for gi in range(HG):
                        g = g0 + gi
                        # v = (e - Z) * (-1/Z) = (Z - e)/Z  >= 0
                        ts_eng = (nc.gpsimd if (POOL_TS and gi % 2 == 0)
                                  else nc.vector)
                        ts_eng.tensor_scalar(
                            out=vs[:, g, :], in0=es[:, g, :],
                            scalar1=zh[:, gi:gi + 1], scalar2=nr[:, gi:gi + 1],
                            op0=OP.subtract, op1=OP.mult)
                # L2 = ln(v)
                nc.scalar.activation(ws[:, csl, :], vs[:, csl, :], AF.Ln)
                # w = v^(1/T) = exp(L2/T)
                nc.scalar.activation(vs[:, csl, :], ws[:, csl, :], AF.Exp,
                                     scale=INV_T)
                # e *= w, fused with next Z row-sums
                if USE_TTR == 1:
                    for gi in range(HG):
                        g = g0 + gi
                        nc.vector.tensor_tensor_reduce(
                            out=es[:, g, :], in0=es[:, g, :], in1=vs[:, g, :],
                            scale=1.0, scalar=0.0, op0=OP.mult, op1=OP.add,
                            accum_out=zh[:, gi:gi + 1])
                elif USE_TTR == 2:
                    # custom-DVE fused multiply+rowsum: out=(e*1+0)*w, accum
                    for gi in range(HG):
                        g = g0 + gi
                        nc.vector.affine_mul_reduce(
                            out=es[:, g, :], accum_out=zh[:, gi:gi + 1],
                            in0=es[:, g, :], in1=vs[:, g, :],
                            scale=1.0, bias=0.0)
                else:
                    nc.vector.tensor_mul(
                        es[:, csl, :], es[:, csl, :], vs[:, csl, :])
                    nc.vector.tensor_reduce(
                        zh[:, :], es[:, csl, :], axis=mybir.AxisListType.X,
                        op=OP.add)

        if "e_end" in tap_aps:
            _dma_out_groups(nc, tap_aps["e_end"], es)

        # ---- finale: out = (exp(T*ln(e) + h) - eps) * x ----
        # clamp ln(0) = -inf to a large finite value (exp still underflows
        # to exactly 0) so no infs ever hit memory. Per-half so ACT/DVE/DMA
        # pipeline across the two halves.
        for half in range(2):
            csl = slice(half * (G // 2), (half + 1) * (G // 2))
            nc.scalar.activation(vs[:, csl, :], es[:, csl, :], AF.Ln)
            nc.vector.tensor_scalar_max(vs[:, csl, :], vs[:, csl, :], -1e30)
            nc.vector.scalar_tensor_tensor(
                out=ws[:, csl, :], in0=vs[:, csl, :],
                scalar=float(np.float32(TEMP)),
                in1=hs[:, csl, :], op0=OP.mult, op1=OP.add)
            nc.scalar.activation(vs[:, csl, :], ws[:, csl, :], AF.Exp)
            nc.vector.scalar_tensor_tensor(
                out=ws[:, csl, :], in0=vs[:, csl, :], scalar=-float(EPS),
                in1=xs[:, csl, :], op0=OP.add, op1=OP.mult)
            for g in range(half * (G // 2), (half + 1) * (G // 2)):
                nc.sync.dma_start(out=out_d[g * P:(g + 1) * P, :],
                                  in_=ws[:, g, :])


def kernel(x, W1, b1, W2, b2):
    x = np.ascontiguousarray(np.asarray(x, dtype=np.float32))
    W1 = np.ascontiguousarray(np.asarray(W1, dtype=np.float32))
    b1 = np.asarray(b1, dtype=np.float32).reshape(HID, 1)
    W2 = np.asarray(W2, dtype=np.float32)
    b2 = np.asarray(b2, dtype=np.float32)
    w2b = np.ascontiguousarray(
        np.concatenate([W2, b2[None, :]], axis=0))  # [65, 640]

    if "nc" not in _CACHE:
        _CACHE["nc"] = _build_nc()
    nc = _CACHE["nc"]

    in_maps = []
    for c in range(N_CORES):
        in_maps.append({
            "x": np.ascontiguousarray(x[c * R:(c + 1) * R, :]),
            "w1": W1,
            "b1": np.ascontiguousarray(b1),
            "w2b": w2b,
        })

    trace = bool(_CACHE.get("trace", False))
    res = run_bass_kernel_spmd(
        nc, in_maps, core_ids=list(range(N_CORES)), trace=trace)
    _CACHE["last_results"] = res
    out = np.concatenate([r["out"] for r in res.results], axis=0)
    return out


# revision 35
# speedup vs baseline: 1.0128x; 1.0128x over previous
"""Trainium2 Bass kernel for nn_DimMasking (iterative softmax top-k masking).

Full-input contract: kernel(**inputs) takes the unsharded inputs
(x [8192,640], W1 [640,64], b1 [64], W2 [64,640], b2 [640]) and returns the
full [8192,640] output. Internally: pure data parallel over the batch dim —
8 shards of 1024 rows, one per NeuronCore; MLP weights replicated.

Math: exp-domain reformulation of the reference scan (validated to ~1e-6
absmax-relative vs the fp32 reference in numpy, ~2e-3 on HW splines):
    h  = relu(x@W1 + b1)@W2 + b2
    e  = exp((log(1+eps) - h)/T);  Z = rowsum(e)
    repeat 64x:
        v = (e - Z) * (-1/Z)     # = 1 - softmax-prob, >= 0   [DVE ts]
        w = v^(1/T)              # ln + exp(scale=1/T)        [2 ACT passes]
        e *= w;  Z = rowsum(e)   # fused custom-DVE mul+reduce [DVE]
    out = (exp(T*ln(e) + h) - eps) * x
which is exactly m' = m*(1 - softmax((log(m+eps)-h)/T)) expressed on
e = ((m+eps)^(1/T))*exp(-h/T), so the rowwise softmax never needs a
separate normalize, max-subtract, or per-iteration log of the state.

Per iteration the two 2560-wide ACT transcendental passes per half
(4 total) are the bottleneck; two independent half-streams keep ACT
saturated. ~701us/core predicted by the cost-model timeline sim;
~790us/core measured on HW via in-NEFF loop repetition deltas.
"""

import numpy as np

import concourse.tile as tile
from concourse import bacc, masks, mybir
from concourse.bass_utils import run_bass_kernel_spmd

# Pin the ACT spline-table set to natural_log_exp_and_others (it contains
# every function this kernel uses: Exp, Ln, Copy, Identity, Relu). Without
# this, the table-load-insertion pass alternates between the exp-only and
# ln-only sets, paying a ~1.3us table reload twice per masking iteration.
_orig_get_tables = bacc.get_activation_tables


def _pinned_get_tables(module_arch):
    tables = _orig_get_tables(module_arch)
    combined = tables.get("natural_log_exp_and_others")
    needed = {
        mybir.ActivationFunctionType.Exp,
        mybir.ActivationFunctionType.Ln,
        mybir.ActivationFunctionType.Copy,
        mybir.ActivationFunctionType.Identity,
        mybir.ActivationFunctionType.Relu,
    }
    if not combined or not needed.issubset(combined):
        return tables  # fall back to default selection
    pinned = {}
    for name, fns in tables.items():
        pinned[name] = fns if name == "natural_log_exp_and_others" else set()
    return pinned


import os as _os
PIN_ACT = _os.environ.get("NO_PIN_ACT", "0") != "1"
USE_TTR = int(_os.environ.get("TTR_MODE", "2"))  # 1=isa-ttr 2=custom-dve 0=plain
POOL_TS = _os.environ.get("POOL_TS", "0") == "1"
PER_GROUP_RECIP = _os.environ.get("PGR", "0") == "1"

F32 = mybir.dt.float32
AF = mybir.ActivationFunctionType
OP = mybir.AluOpType

N_CORES = 8
B = 8192
D = 640          # 5 chunks of 128
HID = 64
R = B // N_CORES  # 1024 rows per core
P = 128
G = R // P        # 8 row-groups per core
DC = D // P       # 5 dim-chunks
N_ITER = 64
TEMP = 0.07
EPS = 1e-7
C0 = float(np.log1p(np.float32(EPS)) / np.float32(TEMP))
INV_T = float(np.float32(1.0) / np.float32(TEMP))

# module-level cache: build/compile once per process
_CACHE = {}


def _build_nc(n_iter=N_ITER, num_devices=N_CORES, taps=(), reps=1):
    nc = bacc.Bacc(
        "TRN2",
        target_bir_lowering=False,
        debug=False,
        enable_asserts=False,
        num_devices=num_devices,
    )
    x_d = nc.dram_tensor("x", [R, D], F32, kind="ExternalInput").ap()
    w1_d = nc.dram_tensor("w1", [D, HID], F32, kind="ExternalInput").ap()
    b1_d = nc.dram_tensor("b1", [HID, 1], F32, kind="ExternalInput").ap()
    w2b_d = nc.dram_tensor("w2b", [HID + 1, D], F32, kind="ExternalInput").ap()
    out_d = nc.dram_tensor("out", [R, D], F32, kind="ExternalOutput").ap()
    tap_aps = {
        name: nc.dram_tensor(f"tap_{name}", [R, D], F32, kind="ExternalOutput").ap()
        for name in taps
    }

    with tile.TileContext(nc) as tc:
        _emit(tc, out_d, x_d, w1_d, b1_d, w2b_d, n_iter=n_iter, tap_aps=tap_aps,
              reps=reps)
    # Scope the activation-table pin strictly to compiling OUR module.
    saved = bacc.get_activation_tables
    try:
        if PIN_ACT:
            bacc.get_activation_tables = _pinned_get_tables
        nc.compile()
    finally:
        bacc.get_activation_tables = saved
    return nc


def _dma_out_groups(nc, dram_ap, sbuf_tile):
    for g in range(G):
        nc.sync.dma_start(out=dram_ap[g * P:(g + 1) * P, :], in_=sbuf_tile[:, g, :])


def _emit(tc, out_d, x_d, w1_d, b1_d, w2b_d, n_iter=N_ITER, tap_aps=None,
          reps=1):
    nc = tc.nc
    from contextlib import ExitStack

    ctx = ExitStack()
    with ctx:
        singles = ctx.enter_context(tc.tile_pool(name="singles", bufs=1))
        zpool = ctx.enter_context(tc.tile_pool(name="zpool", bufs=4))

        # persistent SBUF tensors
        xs = singles.tile([P, G, D], F32)    # x, rows-on-partitions
        xt = singles.tile([P, DC, R], F32)   # x transposed (d-on-partitions)
        hs = singles.tile([P, G, D], F32)    # MLP output h
        es = singles.tile([P, G, D], F32)    # state e = exp(logits)
        # v/w and L2 scratch, double-buffered by iteration parity so that
        # iteration t+1's writes never WAR-serialize against iteration t's
        # reads (removes cross-iteration semaphore chains on HW)
        vs0 = singles.tile([P, G, D], F32)
        vs1 = singles.tile([P, G, D], F32)
        ws0 = singles.tile([P, G, D], F32)
        ws1 = singles.tile([P, G, D], F32)
        vs_pp = [vs0, vs1]
        ws_pp = [ws0, ws1]
        zh_a = singles.tile([P, G // 2], F32)
        zh_b = singles.tile([P, G // 2], F32)
        zhalf = [zh_a, zh_b]
        w1s = singles.tile([P, DC, HID], F32)
        b1s = singles.tile([HID, 1], F32)
        w2bs = singles.tile([HID + 1, D], F32)
        h1r = singles.tile([HID + 1, R], F32)  # relu(x@W1+b1).T with ones row
        ident = singles.tile([P, P], F32)
        c0s = singles.tile([P, 1], F32)        # bias constant C0 for init exp
        nc.vector.memset(c0s[:, :], C0)

        # ---- input DMAs ----
        for g in range(G):
            nc.sync.dma_start(out=xs[:, g, :], in_=x_d[g * P:(g + 1) * P, :])
        nc.sync.dma_start(out=w1s[:, :, :],
                          in_=w1_d.rearrange("(c p) j -> p c j", p=P))
        nc.sync.dma_start(out=b1s[:, :], in_=b1_d[:, :])
        nc.sync.dma_start(out=w2bs[:, :], in_=w2b_d[:, :])

        masks.make_identity(nc, ident[:, :])

        # ---- transpose x: 40 PE transposes of [128,128] blocks.
        # 4 transposes share one PSUM bank, evacuated with a single wide copy
        # (alternating DVE/ACT) to halve evacuation time.
        with tc.tile_pool(name="tp_psum", bufs=3, space="PSUM") as tpp:
            for c in range(DC):
                for gq in range(G // 4):
                    tp = tpp.tile([P, 4 * P], F32)
                    for gj in range(4):
                        g = gq * 4 + gj
                        nc.tensor.transpose(
                            tp[:, gj * P:(gj + 1) * P],
                            xs[:, g, c * P:(c + 1) * P], ident[:, :])
                    dst = xt[:, c, gq * 4 * P:(gq + 1) * 4 * P]
                    if (c + gq) % 2 == 0:
                        nc.vector.tensor_copy(dst, tp[:, :])
                    else:
                        nc.scalar.copy(dst, tp[:, :])

        # ---- MLP matmul 1: h1T[j, r] = sum_d W1[d,j] * xT[d,r] ----
        NH = 2  # split R into halves of 512 (f32 moving max)
        with tc.tile_pool(name="mm1_psum", bufs=2, space="PSUM") as mp1:
            for nh in range(NH):
                ph1 = mp1.tile([HID, R // NH], F32, tag="ph1")
                for c in range(DC):
                    nc.tensor.matmul(
                        ph1[:, :],
                        w1s[:, c, :],
                        xt[:, c, nh * 512:(nh + 1) * 512],
                        start=(c == 0),
                        stop=(c == DC - 1),
                    )
                # relu(+bias) straight out of PSUM
                nc.scalar.activation(
                    h1r[0:HID, nh * 512:(nh + 1) * 512], ph1[:, :],
                    AF.Relu, bias=b1s[:, 0:1], scale=1.0)
        nc.vector.memset(h1r[HID:HID + 1, :], 1.0)

        # ---- MLP matmul 2 + evac: h = h1r.T @ W2b  (bias via ones row) ----
        # Evac does double duty: keep h for the finale, and initialize the
        # loop state e0 = exp(-h/T + C0) with per-group row-sums Z0.
        with tc.tile_pool(name="mm2_psum", bufs=2, space="PSUM") as mp2:
            for g in range(G):
                ph = mp2.tile([P, D], F32, tag="ph")
                lhs = h1r[:, g * P:(g + 1) * P]
                nc.tensor.matmul(ph[:, 0:512], lhs, w2bs[:, 0:512],
                                 start=True, stop=True)
                nc.tensor.matmul(ph[:, 512:D], lhs, w2bs[:, 512:D],
                                 start=True, stop=True)
                nc.vector.tensor_copy(hs[:, g, :], ph[:, :])
                nc.scalar.activation(es[:, g, :], ph[:, :], AF.Exp,
                                     bias=c0s[:, 0:1], scale=-INV_T,
                                     accum_out=zhalf[g // (G // 2)][
                                         :, g % (G // 2):g % (G // 2) + 1])

        if tap_aps is None:
            tap_aps = {}
        if "h" in tap_aps:
            _dma_out_groups(nc, tap_aps["h"], hs)

        # ---- the masking loop (state: e, Z) ----
        HG = G // 2  # 4 groups per half-stream
        for it in range(n_iter * reps):
            vs = vs_pp[it % 2]
            ws = ws_pp[it % 2]
            for half in range(2):
                g0 = half * HG
                csl = slice(g0, g0 + HG)
                zh = zhalf[half]
                nzh = zpool.tile([P, HG], F32, tag="nzh")
                nr = zpool.tile([P, HG], F32, tag="nr")
                if PER_GROUP_RECIP:
                    # per-group negate+recip+v so the first v tiles are ready
                    # as soon as their own Z lands (shorter DVE->ACT chain)
                    for gi in range(HG):
                        g = g0 + gi
                        nc.vector.tensor_scalar_mul(
                            nzh[:, gi:gi + 1], zh[:, gi:gi + 1], -1.0)
                        nc.vector.reciprocal(
                            nr[:, gi:gi + 1], nzh[:, gi:gi + 1])
                        nc.vector.tensor_scalar(
                            out=vs[:, g, :], in0=es[:, g, :],
                            scalar1=zh[:, gi:gi + 1], scalar2=nr[:, gi:gi + 1],
                            op0=OP.subtract, op1=OP.mult)
                else:
                    nc.vector.tensor_scalar_mul(nzh[:, :], zh[:, :], -1.0)
                    nc.vector.reciprocal(nr[:, :], nzh[:, :])  # -1/Z
                    for gi in range(HG):
                        g = g0 + gi
                        # v = (e - Z) * (-1/Z) = (Z - e)/Z  >= 0
                        ts_eng = (nc.gpsimd if (POOL_TS and gi % 2 == 0)
                                  else nc.vector)
                        ts_eng.tensor_scalar(
                            out=vs[:, g, :], in0=es[:, g, :],
                            scalar1=zh[:, gi:gi + 1], scalar2=nr[:, gi:gi + 1],
                            op0=OP.subtract, op1=OP.mult)
                # L2 = ln(v)
                nc.scalar.activation(ws[:, csl, :], vs[:, csl, :], AF.Ln)
                # w = v^(1/T) = exp(L2/T)
                nc.scalar.activation(vs[:, csl, :], ws[:, csl, :], AF.Exp,
                                     scale=INV_T)
                # e *= w, fused with next Z row-sums
                if USE_TTR == 1:
                    for gi in range(HG):
                        g = g0 + gi
                        nc.vector.tensor_tensor_reduce(
                            out=es[:, g, :], in0=es[:, g, :], in1=vs[:, g, :],
                            scale=1.0, scalar=0.0, op0=OP.mult, op1=OP.add,
                            accum_out=zh[:, gi:gi + 1])
                elif USE_TTR == 2:
                    # custom-DVE fused multiply+rowsum: out=(e*1+0)*w, accum
                    for gi in range(HG):
                        g = g0 + gi
                        nc.vector.affine_mul_reduce(
                            out=es[:, g, :], accum_out=zh[:, gi:gi + 1],
                            in0=es[:, g, :], in1=vs[:, g, :],
                            scale=1.0, bias=0.0)
                else:
                    nc.vector.tensor_mul(
                        es[:, csl, :], es[:, csl, :], vs[:, csl, :])
                    nc.vector.tensor_reduce(
                        zh[:, :], es[:, csl, :], axis=mybir.AxisListType.X,
                        op=OP.add)

        if "e_end" in tap_aps:
            _dma_out_groups(nc, tap_aps["e_end"], es)
        vs = vs_pp[0]
        ws = ws_pp[0]

        # ---- finale: out = (exp(T*ln(e) + h) - eps) * x ----
        # clamp ln(0) = -inf to a large finite value (exp still underflows
        # to exactly 0) so no infs ever hit memory. Per-half so ACT/DVE/DMA
        # pipeline across the two halves.
        for half in range(2):
            csl = slice(half * (G // 2), (half + 1) * (G // 2))
            nc.scalar.activation(vs[:, csl, :], es[:, csl, :], AF.Ln)
            nc.vector.tensor_scalar_max(vs[:, csl, :], vs[:, csl, :], -1e30)
            nc.vector.scalar_tensor_tensor(
                out=ws[:, csl, :], in0=vs[:, csl, :],
                scalar=float(np.float32(TEMP)),
                in1=hs[:, csl, :], op0=OP.mult, op1=OP.add)
            nc.scalar.activation(vs[:, csl, :], ws[:, csl, :], AF.Exp)
            nc.vector.scalar_tensor_tensor(
                out=ws[:, csl, :], in0=vs[:, csl, :], scalar=-float(EPS),
                in1=xs[:, csl, :], op0=OP.add, op1=OP.mult)
            for g in range(half * (G // 2), (half + 1) * (G // 2)):
                nc.sync.dma_start(out=out_d[g * P:(g + 1) * P, :],
                                  in_=ws[:, g, :])


def kernel(x, W1, b1, W2, b2):
    x = np.ascontiguousarray(np.asarray(x, dtype=np.float32))
    W1 = np.ascontiguousarray(np.asarray(W1, dtype=np.float32))
    b1 = np.asarray(b1, dtype=np.float32).reshape(HID, 1)
    W2 = np.asarray(W2, dtype=np.float32)
    b2 = np.asarray(b2, dtype=np.float32)
    w2b = np.ascontiguousarray(
        np.concatenate([W2, b2[None, :]], axis=0))  # [65, 640]

    if "nc" not in _CACHE:
        _CACHE["nc"] = _build_nc()
    nc = _CACHE["nc"]

    in_maps = []
    for c in range(N_CORES):
        in_maps.append({
            "x": np.ascontiguousarray(x[c * R:(c + 1) * R, :]),
            "w1": W1,
            "b1": np.ascontiguousarray(b1),
            "w2b": w2b,
        })

    trace = bool(_CACHE.get("trace", False))
    res = run_bass_kernel_spmd(
        nc, in_maps, core_ids=list(range(N_CORES)), trace=trace)
    _CACHE["last_results"] = res
    out = np.concatenate([r["out"] for r in res.results], axis=0)
    return out
